# revision 33
# baseline (speedup 1.0000x reference)
"""Multi-head self-attention (B=2, S=2048, E=1024, H=16, D=64) on 8 NeuronCores.

Sharding: core c -> (batch b = c // 4, head group g = c % 4).  Each core
computes Q/K/V projections for its 4 heads (column-parallel), attention, and
a partial output projection (row-parallel); the host sums the 4 partials per
batch.  All device activations live in "transposed space" (feature on the
partition dim) so every matmul contracts along partitions with no on-device
transposes:

  Q^T = Wq_g^T @ X^T          [256, 2048]  (e-chunk accumulated; bias via DVE)
  K^T = Wk_g^T @ X^T          [256, 2048]
  V   = X @ Wv_g              [2048, 256]  (natural; ones column appended)
  S^T = K_h @ Q_h^T / 8       [2048, 2048] per head (computed tile-wise)
  P^T = exp(S^T)              (softmax without max-subtraction: scores ~N(0,1))
  O'^T = [V_h | 1]^T @ P^T    [65, q]  (row 64 = softmax denominators)
  O^T  = O'[0:64] / O'[64]    (DVE reciprocal + GpSimd partition broadcast)
  Y^T  = Wo_g^T @ O^T         [1024, 2048] partial, host-summed per batch

bv and bo are folded on the host (exact: softmax rows sum to 1, so
attn(V + bv) = attn(V) + bv, and the output projection is linear).

Schedule: the exp stream (ScalarE) and the matmul stream (PE) are both near
their engine floors (~128us and ~140us), so the emission order software-
pipelines them: per k-chunk the PE emits the next scores pair + the previous
block's PV accumulation + one "filler" (projection / output chunk) sized to
keep PE just under the ACT rate.  Warmup matmuls + a dummy exp run during the
initial DMA so the PE starts HAM-warm and the exp table set is preloaded.
"""

from contextlib import ExitStack

import numpy as np

import concourse.bass as bass
import concourse.tile as tile
from concourse import bacc, mybir
from concourse.bass_utils import run_bass_kernel_spmd

B, S, E, H, D = 2, 2048, 1024, 16, 64
NCORES = 8
GH = 4            # heads per core
DC = GH * D       # head-dim columns per core (256)
EC = E // 128     # 8 e-chunks
KC = S // 128     # 16 k-chunks
F32 = mybir.dt.float32
MM_DT = mybir.dt.float16    # full-speed 16-bit matmul path (10-bit mantissa)
EXP_FUNC = mybir.ActivationFunctionType.Exp
ADD = mybir.AluOpType.add
SCALE = 1.0 / np.sqrt(np.float32(D))


def round_f32r(a):
    # Host-side conversion to the matmul dtype (RNE)
    if MM_DT == mybir.dt.float16:
        return np.ascontiguousarray(a, np.float32).astype(np.float16)
    if MM_DT == mybir.dt.bfloat16:
        import ml_dtypes
        return np.ascontiguousarray(a, np.float32).astype(ml_dtypes.bfloat16)
    return np.ascontiguousarray(a, np.float32)


DEBUG_DUMPS = False


def _emit(nc, tc, ctx, xT, wq, wk, wv, wo, bqk, yT, dbg=None):
    sb_big = ctx.enter_context(tc.tile_pool(name="sb_big", bufs=1))
    sb_p = ctx.enter_context(tc.tile_pool(name="sb_p", bufs=28))
    sb_norm = ctx.enter_context(tc.tile_pool(name="sb_norm", bufs=4))
    sb_y = ctx.enter_context(tc.tile_pool(name="sb_y", bufs=4))
    ps_sco = ctx.enter_context(tc.tile_pool(name="ps_sco", bufs=2, space="PSUM"))
    ps_acc = ctx.enter_context(tc.tile_pool(name="ps_acc", bufs=4, space="PSUM"))

    xT_t = sb_big.tile([128, 4, EC, 512], MM_DT)
    wq_t = sb_big.tile([128, 2, EC, 128], MM_DT)
    wk_t = sb_big.tile([128, 2, EC, 128], MM_DT)
    wv_t = sb_big.tile([128, EC, DC], MM_DT)
    wo_t = sb_big.tile([128, 2, E], MM_DT)
    bqk_t = sb_big.tile([128, 4], F32)
    qT_t = sb_big.tile([128, 2, S], MM_DT)
    kT_t = sb_big.tile([128, 2, S], MM_DT)
    v_t = sb_big.tile([128, KC, GH, D + 1], MM_DT)
    o_t = sb_big.tile([128, 2, S], MM_DT)
    warm_t = sb_big.tile([128, 512], MM_DT)
    warm_o = sb_big.tile([128, 512], MM_DT)

    # --- warmup: PE busy + exp table preload while input DMAs run -------
    nc.vector.memset(warm_t[:, :], 0.125)
    nc.scalar.activation(out=warm_o[:, :], in_=warm_t[:, :], func=EXP_FUNC,
                         scale=float(SCALE))
    gate_t = sb_big.tile([1, 128], F32)
    warm_a = ps_acc.tile([128, 512], F32, tag="acc", name="warm_a")
    for i in range(6):
        nc.tensor.matmul(warm_a[:, :], lhsT=warm_t[:, 0:128],
                         rhs=warm_t[:, :], start=True, stop=True)
    # gate A fires when warmup MM 6 completes (~11us): releases x(sc1)
    nc.vector.tensor_copy(out=gate_t[0:1, 0:8], in_=warm_a[0:1, 0:8])
    warm_b = ps_acc.tile([128, 512], F32, tag="acc", name="warm_b")
    for i in range(4):
        nc.tensor.matmul(warm_b[:, :], lhsT=warm_t[:, 0:128],
                         rhs=warm_t[:, :], start=True, stop=True)
    # gate B (~13us): releases x(sc2), x(sc3), wv
    nc.vector.tensor_copy(out=gate_t[0:1, 16:24], in_=warm_b[0:1, 0:8])
    # readers: create WAR deps so the gated DMAs start only after their
    # gate — keeps the first-needed transfers from sharing HBM bandwidth
    nc.vector.tensor_mul(gate_t[0:1, 8:16], gate_t[0:1, 0:8],
                         xT_t[0:1, 1, 0, 0:8])
    nc.vector.tensor_mul(gate_t[0:1, 24:32], gate_t[0:1, 16:24],
                         xT_t[0:1, 2, 0, 0:8])
    nc.vector.tensor_mul(gate_t[0:1, 32:40], gate_t[0:1, 16:24],
                         xT_t[0:1, 3, 0, 0:8])
    nc.vector.tensor_mul(gate_t[0:1, 40:48], gate_t[0:1, 16:24],
                         wv_t[0:1, 0, 0:8])

    # --- input DMAs: priority-ordered across 3 queues -------------------
    # sync carries x (the critical path to the first scores); gpsimd the
    # early weights; scalar (after the dummy exp) the output projection.
    wq_r = wq.rearrange("p (dc c d) -> p dc c d", dc=2, c=EC)
    wk_r = wk.rearrange("p (dc c d) -> p dc c d", dc=2, c=EC)
    x_r = xT.rearrange("p (sc c s) -> p sc c s", sc=4, c=EC)
    wo_r = wo.rearrange("p (c e) -> p c e", c=2)
    nc.sync.dma_start(out=xT_t[:, 0, 0:4, :], in_=x_r[:, 0, 0:4, :])
    nc.sync.dma_start(out=xT_t[:, 0, 4:8, :], in_=x_r[:, 0, 4:8, :])
    for sc in range(1, 4):
        nc.sync.dma_start(out=xT_t[:, sc, :, :], in_=x_r[:, sc, :, :])
    nc.gpsimd.dma_start(out=bqk_t[:, :], in_=bqk)
    nc.gpsimd.dma_start(out=wk_t[:, 0, :, :], in_=wk_r[:, 0, :, :])
    nc.gpsimd.dma_start(out=wq_t[:, 0, :, :], in_=wq_r[:, 0, :, :])
    nc.gpsimd.dma_start(out=wv_t[:, :, :], in_=wv.rearrange(
        "p (c d) -> p c d", c=EC))
    for kc in range(KC):
        nc.vector.memset(v_t[:, kc, :, D:D + 1], 1.0)

    def qk_part(dc, proj, sc, half, state={}):
        # psum[d, s] += W[e, d].T @ X^T[e, s], two halves so bursts stay
        # small; bias folded into the DVE evacuation copy.
        w_t, dst = ((wq_t, qT_t), (wk_t, kT_t))[proj]
        if half == 0:
            state[(dc, proj, sc)] = ps_acc.tile(
                [128, 512], F32, tag="acc", name="ps_qk")
        ps = state[(dc, proj, sc)]
        ecs = range(EC // 2) if half == 0 else range(EC // 2, EC)
        for ec in ecs:
            nc.tensor.matmul(
                ps[:, :],
                lhsT=w_t[:, dc, ec, :],
                rhs=xT_t[:, sc, ec, :],
                start=(ec == 0), stop=(ec == EC - 1))
        if half == 1:
            nc.vector.tensor_scalar(
                out=dst[:, dc, sc * 512:(sc + 1) * 512], in0=ps[:, :],
                scalar1=bqk_t[:, 2 * proj + dc:2 * proj + dc + 1],
                scalar2=None, op0=ADD)
            del state[(dc, proj, sc)]

    def v_part(kc, half, state={}):
        # psum[s, d] += X^T[e, s].T @ Wv[e, d]
        if half == 0:
            state[kc] = ps_acc.tile([128, 512], F32, tag="acc", name="ps_v")
        ps = state[kc]
        sc, ko = kc // 4, (kc % 4) * 128
        ecs = range(EC // 2) if half == 0 else range(EC // 2, EC)
        for ec in ecs:
            nc.tensor.matmul(
                ps[:, 0:DC],
                lhsT=xT_t[:, sc, ec, ko:ko + 128],
                rhs=wv_t[:, ec, :],
                start=(ec == 0), stop=(ec == EC - 1))
        if half == 1:
            nc.vector.tensor_copy(
                out=v_t[:, kc, :, 0:D],
                in_=ps[:, 0:DC].rearrange("p (h d) -> p h d", h=GH))
            del state[kc]

    def attention_scores(qc, hc, kc):
        # Head pair (2*hc, 2*hc+1): head hp=0 on SBUF partitions 0-63, hp=1
        # on 64-127, so the two scores matmuls run as independent 64x128 PE
        # tiles and one ACTIVATE covers both heads' exp.
        sco = ps_sco.tile([128, 2, 512], F32, tag="sco", name="sco")
        for hp in range(2):
            po = hp * 64
            nc.tensor.matmul(
                sco[:, hp, :],
                lhsT=kT_t[po:po + 64, hc, kc * 128:(kc + 1) * 128],
                rhs=qT_t[po:po + 64, hc, qc * 512:(qc + 1) * 512],
                start=True, stop=True)
        pT = sb_p.tile([128, 2, 512], MM_DT)
        nc.scalar.activation(
            out=pT[:, :, :], in_=sco[:, :, :], func=EXP_FUNC,
            scale=float(SCALE))
        return pT

    def pv_alloc():
        return [ps_acc.tile([128, 512], F32, tag="acc", name=f"acc{j}")
                for j in range(2)]

    def pv_kc(accs, hc, pTs, kc):
        for hp in range(2):
            h = 2 * hc + hp
            nc.tensor.matmul(
                accs[hp][0:D + 1, :],
                lhsT=v_t[:, kc, h, :],
                rhs=pTs[kc][:, hp, :],
                start=(kc == 0), stop=(kc == KC - 1))

    def attention_norm(qc, hc, accs):
        # evacuate both PSUM accumulators first: their banks then free
        # ~4us earlier (the norm mul otherwise holds them), so the next
        # block's PV allocation never stalls the in-order PE queue.
        evs = []
        for hp in range(2):
            ev = sb_norm.tile([64, 512], F32, tag="brd")
            nc.vector.tensor_copy(out=ev[:, :], in_=accs[hp][0:D, :])
            rs = sb_norm.tile([1, 512], F32, tag="rs")
            nc.vector.tensor_copy(out=rs[:, :], in_=accs[hp][D:D + 1, :])
            evs.append((ev, rs))
        for hp in range(2):
            po = hp * 64
            ev, rs = evs[hp]
            inv_r = sb_norm.tile([1, 512], F32, tag="inv")
            nc.vector.reciprocal_approx_fast(out=inv_r[:, :], in_=rs[:, :])
            brd = sb_norm.tile([64, 512], F32, tag="brd")
            nc.gpsimd.partition_broadcast(brd[:, :], inv_r[:, :])
            nc.vector.tensor_mul(
                o_t[po:po + 64, hc, qc * 512:(qc + 1) * 512],
                ev[:, :],
                brd[:, :])

    def y_group(qc, ec, copy_eng=None):
        # psum[e, s] += Wo[c, e].T @ O^T[c, s] for chunk (ec, qc)
        yp = ps_acc.tile([128, 512], F32, tag="acc", name="yp")
        for cc in range(2):
            nc.tensor.matmul(
                yp[:, :],
                lhsT=wo_t[:, cc, ec * 128:(ec + 1) * 128],
                rhs=o_t[:, cc, qc * 512:(qc + 1) * 512],
                start=(cc == 0), stop=(cc == 1))
        ys = sb_y.tile([128, 512], F32)
        if copy_eng == "scalar":
            nc.scalar.copy(out=ys[:, :], in_=yp[:, :])
        else:
            nc.vector.tensor_copy(out=ys[:, :], in_=yp[:, :])
        nc.sync.dma_start(
            out=yT[ec * 128:(ec + 1) * 128, qc * 512:(qc + 1) * 512],
            in_=ys[:, :])

    # --- software-pipelined emission (= static engine program order) ----
    # The Tile scheduler orders instructions statically and every engine
    # executes its queue IN ORDER, so emission layout IS the schedule: a
    # not-ready instruction head-of-line-blocks its whole engine.  Per kc
    # slot the PE gets: the scores pair (feeds ACT), PV units per the
    # global PV plan, and fillers placed no earlier than their data.
    #
    # PV plan: block b's PV runs as "self" units (kc 0..5 in its own slots
    # 10..15, lag >= 2 behind exp) plus "leftover" units (kc 6..15
    # compressed into the next block's slots 0..7, norm right after), so
    # the final tail holds only 2 kc of PV + norm + y3.  Blocks 0-2 keep
    # the simple spread (block 0/1 are loaded with v/projection fillers).
    blocks = [(0, 0), (1, 0), (0, 1), (1, 1), (2, 0), (2, 1), (3, 0), (3, 1)]

    pts_of = [[] for _ in range(8)]
    accs_of = {}

    def pv_unit(b, kc):
        if kc == 0:
            accs_of[b] = pv_alloc()
        pv_kc(accs_of[b], blocks[b][1], pts_of[b], kc)
        if kc == KC - 1:
            attention_norm(blocks[b][0], blocks[b][1], accs_of.pop(b))

    # pv_sched[bi][slot] = list of (owner_block, kc) units.  Uniform: one
    # unit per slot — block b runs its own kc 0..5 in slots 10..15 (lag
    # >= 2 behind its exp stream) and kc 6..15 in the next block's slots
    # 0..9 (norm lands at slot 9); the last block self-runs kc 0..13 at
    # lag 2 so the tail holds only 2 PV units + norm + y3.
    pv_sched = [dict() for _ in range(8)]
    pv_sched[0] = {10 + k: [(0, k)] for k in range(6)}
    for bi in range(1, 7):
        sched = {s: [(bi - 1, 6 + s)] for s in range(10)}
        sched.update({10 + k: [(bi, k)] for k in range(6)})
        pv_sched[bi] = sched
    sched = {s: [(6, 6 + s)] for s in range(10)}
    for k in range(14):                      # self, lag 2
        sched.setdefault(2 + k, []).append((7, k))
    pv_sched[7] = sched

    def qk(dc, proj, sc, half):
        return lambda: qk_part(dc, proj, sc, half)

    def vp(kc, half):
        return lambda: v_part(kc, half)

    def yg(qc, ec):
        return lambda: y_group(qc, ec)

    # fillers per block: (min_slot, fn); deadlines as emission-order
    # constraints (data must be emitted before its consumer's slot):
    #   K(dc0,s) before block0 kc=4s; Q(dc0,s1) < blk1; K/Q(dc1,s0) < blk2;
    #   K(dc1,s2/s3) before blk2 kc8/kc12; Q(dc1,s1) < blk3; Q(dc0,s2) <
    #   blk4; Q(dc1,s2) < blk5; Q(dc0,s3) < blk6; Q(dc1,s3) < blk7;
    #   v(kc) before its first PV unit; y(qc) after norm(qc,1); min_slots
    #   on early-block items also track DMA arrival order.
    fillers = {
        0: [(2, qk(0, 1, 1, 0)), (3, qk(0, 1, 1, 1)), (4, qk(0, 0, 1, 0)),
            (5, qk(0, 1, 2, 0)), (6, qk(0, 1, 2, 1)), (7, qk(0, 0, 1, 1)),
            (9, qk(0, 1, 3, 0)), (10, qk(0, 1, 3, 1))] +
           [(2 + k, vp(k, 0)) for k in range(10)] +
           [(3 + k, vp(k, 1)) for k in range(10)],
        1: [(0, vp(10, 0)), (1, vp(10, 1)), (1, vp(11, 0)), (2, vp(11, 1)),
            (3, vp(12, 0)), (4, vp(12, 1)), (4, vp(13, 0)), (5, vp(13, 1)),
            (6, vp(14, 0)), (7, vp(14, 1)), (7, vp(15, 0)), (8, vp(15, 1)),
            (9, qk(1, 1, 0, 0)), (10, qk(1, 1, 0, 1)),
            (11, qk(1, 0, 0, 0)), (12, qk(1, 0, 0, 1))],
        2: [(0, qk(1, 1, 1, 0)), (1, qk(1, 1, 1, 1)),
            (2, qk(1, 1, 2, 0)), (3, qk(1, 1, 2, 1)),
            (6, qk(1, 1, 3, 0)), (7, qk(1, 1, 3, 1)),
            (10, qk(1, 0, 1, 0)), (11, qk(1, 0, 1, 1))],
        3: [(0, qk(0, 0, 2, 0)), (1, qk(0, 0, 2, 1))],
        4: [(0, qk(1, 0, 2, 0)), (1, qk(1, 0, 2, 1))] +
           [(0, yg(0, e)) for e in range(EC)],
        5: [(0, qk(0, 0, 3, 0)), (1, qk(0, 0, 3, 1))] +
           [(0, yg(1, e)) for e in range(EC)],
        6: [(0, qk(1, 0, 3, 0)), (1, qk(1, 0, 3, 1))] +
           [(11, yg(2, e)) for e in range(EC)],
        7: [],
    }

    def layout(items):
        # first-fit at the intended slot, spill forward once 2 items deep
        slots = [[] for _ in range(16)]
        for ms, fn in items:
            s = ms
            while s < 15 and len(slots[s]) >= 2:
                s += 1
            slots[s].append(fn)
        return slots

    # block 0's own first tiles; h0 halves first so they start as soon as
    # the low e-chunks of x(sc0) land, h1 halves follow the high chunks
    qk_part(0, 0, 0, 0)
    qk_part(0, 1, 0, 0)
    qk_part(0, 0, 0, 1)
    qk_part(0, 1, 0, 1)
    # gate C (first qT tile copied, ~20us): releases dc1 weights + wo
    nc.vector.tensor_mul(gate_t[0:1, 48:56], qT_t[0:1, 0, 0:8],
                         wq_t[0:1, 1, 0, 0:8])
    nc.vector.tensor_mul(gate_t[0:1, 56:64], qT_t[0:1, 0, 0:8],
                         wk_t[0:1, 1, 0, 0:8])
    nc.vector.tensor_mul(gate_t[0:1, 64:72], qT_t[0:1, 0, 0:8],
                         wo_t[0:1, 0, 0:8])
    nc.vector.tensor_mul(gate_t[0:1, 72:80], qT_t[0:1, 0, 0:8],
                         wo_t[0:1, 1, 0:8])
    nc.gpsimd.dma_start(out=wq_t[:, 1, :, :], in_=wq_r[:, 1, :, :])
    nc.gpsimd.dma_start(out=wk_t[:, 1, :, :], in_=wk_r[:, 1, :, :])
    nc.scalar.dma_start(out=wo_t[:, 0, :], in_=wo_r[:, 0, :])
    nc.scalar.dma_start(out=wo_t[:, 1, :], in_=wo_r[:, 1, :])

    for bi, (qc, hc) in enumerate(blocks):
        fl_slots = layout(fillers.get(bi, []))
        for kc in range(0, KC, 2):
            # scores pairs back-to-back: the second pair's kT LDWEIGHTS
            # overlaps the first pair's matmuls (disjoint PE row groups)
            pts_of[bi].append(attention_scores(qc, hc, kc))
            pts_of[bi].append(attention_scores(qc, hc, kc + 1))
            for s in (kc, kc + 1):
                for fn in fl_slots[s]:
                    fn()
                for b, k in pv_sched[bi].get(s, []):
                    pv_unit(b, k)
    # tail: last 2 PV units + norm + the last y chunks
    pv_unit(7, 14)
    pv_unit(7, 15)
    for ec in range(EC):
        y_group(3, ec, copy_eng="scalar" if ec % 2 else None)

    if dbg is not None:
        for name, t in (("qT", qT_t), ("kT", kT_t), ("o", o_t)):
            f = sb_big.tile([128, 2, S], F32, name=f"dump_{name}")
            nc.vector.tensor_copy(out=f[:, :, :], in_=t[:, :, :])
            nc.sync.dma_start(out=dbg[name], in_=f.rearrange("p a b -> p (a b)"))
        fv = sb_big.tile([128, KC, GH, D + 1], F32, name="dump_v")
        nc.vector.tensor_copy(out=fv[:, :, :, :], in_=v_t[:, :, :, :])
        nc.sync.dma_start(out=dbg["v"], in_=fv.rearrange("p a b c -> p (a b c)"))


_cached_nc = None


def _build():
    nc = bacc.Bacc(trn_type="TRN2", target_bir_lowering=False)
    xT = nc.dram_tensor("xT", [128, EC * S], MM_DT, kind="ExternalInput").ap()
    wq = nc.dram_tensor("wq", [128, EC * DC], MM_DT, kind="ExternalInput").ap()
    wk = nc.dram_tensor("wk", [128, EC * DC], MM_DT, kind="ExternalInput").ap()
    wv = nc.dram_tensor("wv", [128, EC * DC], MM_DT, kind="ExternalInput").ap()
    wo = nc.dram_tensor("wo", [128, 2 * E], MM_DT, kind="ExternalInput").ap()
    bqk = nc.dram_tensor("bqk", [128, 4], F32, kind="ExternalInput").ap()
    yT = nc.dram_tensor("yT", [E, S], F32, kind="ExternalOutput").ap()
    dbg = None
    if DEBUG_DUMPS:
        dbg = {
            "qT": nc.dram_tensor("dbg_qT", [128, 2 * S], F32, kind="ExternalOutput").ap(),
            "kT": nc.dram_tensor("dbg_kT", [128, 2 * S], F32, kind="ExternalOutput").ap(),
            "o": nc.dram_tensor("dbg_o", [128, 2 * S], F32, kind="ExternalOutput").ap(),
            "v": nc.dram_tensor("dbg_v", [128, KC * GH * (D + 1)], F32, kind="ExternalOutput").ap(),
        }
    with tile.TileContext(nc) as tc:
        with ExitStack() as ctx:
            _emit(nc, tc, ctx, xT, wq, wk, wv, wo, bqk, yT, dbg)
    nc.compile()
    return nc


def get_nc():
    global _cached_nc
    if _cached_nc is None:
        _cached_nc = _build()
    return _cached_nc


def make_in_maps(inputs, wq, bq, wk, bk, wv, wo):
    in_maps = []
    for c in range(NCORES):
        b, g = divmod(c, GH)
        sl = slice(g * DC, (g + 1) * DC)
        def perm(a):
            # [C*128, N] -> [128, C*N] with SBUF chunk-major free dim
            cN = a.shape[0] // 128
            return np.ascontiguousarray(
                a.reshape(cN, 128, a.shape[1]).transpose(1, 0, 2).reshape(
                    128, cN * a.shape[1]))

        def perm_x(a):
            # [E, S] -> [128, SC, EC, 512] flattened (sc-major, 8KB runs)
            return np.ascontiguousarray(
                a.reshape(EC, 128, 4, 512).transpose(1, 2, 0, 3).reshape(
                    128, -1))

        def perm_w(a):
            # [E, DC] -> [128, 2, EC, 128] flattened (dc-major, 2KB runs)
            return np.ascontiguousarray(
                a.reshape(EC, 128, 2, 128).transpose(1, 2, 0, 3).reshape(
                    128, -1))

        bq_g, bk_g = bq[sl], bk[sl]
        bqk = np.stack([bq_g[0:128], bq_g[128:256],
                        bk_g[0:128], bk_g[128:256]], axis=1)
        in_maps.append({
            "xT": round_f32r(perm_x(np.ascontiguousarray(inputs[b].T))),
            "wq": round_f32r(perm_w(wq[:, sl])),
            "wk": round_f32r(perm_w(wk[:, sl])),
            "wv": round_f32r(perm(wv[:, sl])),
            "wo": round_f32r(perm(wo[sl, :])),
            "bqk": np.ascontiguousarray(bqk, np.float32),
        })
    return in_maps


def combine(results, wv_full, bv, wo_full, bo):
    y = np.zeros((B, S, E), np.float32)
    for c in range(NCORES):
        y[c // GH] += results[c]["yT"].T
    y += bv @ wo_full + bo
    return y


def kernel(inputs, wq, bq, wk, bk, wv, bv, wo, bo, _run_kwargs=None):
    inputs = np.asarray(inputs, np.float32)
    wq, bq = np.asarray(wq, np.float32), np.asarray(bq, np.float32)
    wk, bk = np.asarray(wk, np.float32), np.asarray(bk, np.float32)
    wv, bv = np.asarray(wv, np.float32), np.asarray(bv, np.float32)
    wo, bo = np.asarray(wo, np.float32), np.asarray(bo, np.float32)

    nc = get_nc()
    in_maps = make_in_maps(inputs, wq, bq, wk, bk, wv, wo)
    res = run_bass_kernel_spmd(nc, in_maps, list(range(NCORES)),
                               **(_run_kwargs or {}))
    y = combine(res.results, wv, bv, wo, bo)
    if _run_kwargs:
        kernel.last_result = res
    return y


# revision 34
# speedup vs baseline: 1.0038x; 1.0038x over previous
"""Multi-head self-attention (B=2, S=2048, E=1024, H=16, D=64) on 8 NeuronCores.

Sharding: core c -> (batch b = c // 4, head group g = c % 4).  Each core
computes Q/K/V projections for its 4 heads (column-parallel), attention, and
a partial output projection (row-parallel); the host sums the 4 partials per
batch.  All device activations live in "transposed space" (feature on the
partition dim) so every matmul contracts along partitions with no on-device
transposes:

  Q^T = Wq_g^T @ X^T          [256, 2048]  (e-chunk accumulated; bias via DVE)
  K^T = Wk_g^T @ X^T          [256, 2048]
  V   = X @ Wv_g              [2048, 256]  (natural; ones column appended)
  S^T = K_h @ Q_h^T / 8       [2048, 2048] per head (computed tile-wise)
  P^T = exp(S^T)              (softmax without max-subtraction: scores ~N(0,1))
  O'^T = [V_h | 1]^T @ P^T    [65, q]  (row 64 = softmax denominators)
  O^T  = O'[0:64] / O'[64]    (DVE reciprocal + GpSimd partition broadcast)
  Y^T  = Wo_g^T @ O^T         [1024, 2048] partial, host-summed per batch

bv and bo are folded on the host (exact: softmax rows sum to 1, so
attn(V + bv) = attn(V) + bv, and the output projection is linear).

Schedule: the exp stream (ScalarE) and the matmul stream (PE) are both near
their engine floors (~128us and ~140us), so the emission order software-
pipelines them: per k-chunk the PE emits the next scores pair + the previous
block's PV accumulation + one "filler" (projection / output chunk) sized to
keep PE just under the ACT rate.  Warmup matmuls + a dummy exp run during the
initial DMA so the PE starts HAM-warm and the exp table set is preloaded.
"""

from contextlib import ExitStack

import numpy as np

import concourse.bass as bass
import concourse.tile as tile
from concourse import bacc, mybir
from concourse.bass_utils import run_bass_kernel_spmd

B, S, E, H, D = 2, 2048, 1024, 16, 64
NCORES = 8
GH = 4            # heads per core
DC = GH * D       # head-dim columns per core (256)
EC = E // 128     # 8 e-chunks
KC = S // 128     # 16 k-chunks
F32 = mybir.dt.float32
MM_DT = mybir.dt.float16    # full-speed 16-bit matmul path (10-bit mantissa)
EXP_FUNC = mybir.ActivationFunctionType.Exp
ADD = mybir.AluOpType.add
SCALE = 1.0 / np.sqrt(np.float32(D))


def round_f32r(a):
    # Host-side conversion to the matmul dtype (RNE)
    if MM_DT == mybir.dt.float16:
        return np.ascontiguousarray(a, np.float32).astype(np.float16)
    if MM_DT == mybir.dt.bfloat16:
        import ml_dtypes
        return np.ascontiguousarray(a, np.float32).astype(ml_dtypes.bfloat16)
    return np.ascontiguousarray(a, np.float32)


DEBUG_DUMPS = False


def _emit(nc, tc, ctx, xT, wq, wk, wv, wo, bqk, yT, dbg=None):
    sb_big = ctx.enter_context(tc.tile_pool(name="sb_big", bufs=1))
    sb_p = ctx.enter_context(tc.tile_pool(name="sb_p", bufs=28))
    sb_norm = ctx.enter_context(tc.tile_pool(name="sb_norm", bufs=4))
    sb_y = ctx.enter_context(tc.tile_pool(name="sb_y", bufs=4))
    ps_sco = ctx.enter_context(tc.tile_pool(name="ps_sco", bufs=2, space="PSUM"))
    ps_acc = ctx.enter_context(tc.tile_pool(name="ps_acc", bufs=4, space="PSUM"))

    xT_t = sb_big.tile([128, 4, EC, 512], MM_DT)
    wq_t = sb_big.tile([128, 2, EC, 128], MM_DT)
    wk_t = sb_big.tile([128, 2, EC, 128], MM_DT)
    wv_t = sb_big.tile([128, EC, DC], MM_DT)
    wo_t = sb_big.tile([128, 2, E], MM_DT)
    bqk_t = sb_big.tile([128, 4], F32)
    qT_t = sb_big.tile([128, 2, S], MM_DT)
    kT_t = sb_big.tile([128, 2, S], MM_DT)
    v_t = sb_big.tile([128, KC, GH, D + 1], MM_DT)
    o_t = sb_big.tile([128, 2, S], MM_DT)
    warm_t = sb_big.tile([128, 512], MM_DT)
    warm_o = sb_big.tile([128, 512], MM_DT)

    # --- warmup: PE busy + exp table preload while input DMAs run -------
    nc.vector.memset(warm_t[:, :], 0.125)
    nc.scalar.activation(out=warm_o[:, :], in_=warm_t[:, :], func=EXP_FUNC,
                         scale=float(SCALE))
    gate_t = sb_big.tile([1, 128], F32)
    warm_a = ps_acc.tile([128, 512], F32, tag="acc", name="warm_a")
    for i in range(6):
        nc.tensor.matmul(warm_a[:, :], lhsT=warm_t[:, 0:128],
                         rhs=warm_t[:, :], start=True, stop=True)
    # gate A fires when warmup MM 6 completes (~11us): releases x(sc1)
    nc.vector.tensor_copy(out=gate_t[0:1, 0:8], in_=warm_a[0:1, 0:8])
    warm_b = ps_acc.tile([128, 512], F32, tag="acc", name="warm_b")
    for i in range(4):
        nc.tensor.matmul(warm_b[:, :], lhsT=warm_t[:, 0:128],
                         rhs=warm_t[:, :], start=True, stop=True)
    # gate B (~13us): releases x(sc2), x(sc3), wv
    nc.vector.tensor_copy(out=gate_t[0:1, 16:24], in_=warm_b[0:1, 0:8])
    # readers: create WAR deps so the gated DMAs start only after their
    # gate — keeps the first-needed transfers from sharing HBM bandwidth
    nc.vector.tensor_mul(gate_t[0:1, 8:16], gate_t[0:1, 0:8],
                         xT_t[0:1, 1, 0, 0:8])
    nc.vector.tensor_mul(gate_t[0:1, 24:32], gate_t[0:1, 16:24],
                         xT_t[0:1, 2, 0, 0:8])
    nc.vector.tensor_mul(gate_t[0:1, 32:40], gate_t[0:1, 16:24],
                         xT_t[0:1, 3, 0, 0:8])
    nc.vector.tensor_mul(gate_t[0:1, 40:48], gate_t[0:1, 16:24],
                         wv_t[0:1, 0, 0:8])

    # --- input DMAs: priority-ordered across 3 queues -------------------
    # sync carries x (the critical path to the first scores); gpsimd the
    # early weights; scalar (after the dummy exp) the output projection.
    wq_r = wq.rearrange("p (dc c d) -> p dc c d", dc=2, c=EC)
    wk_r = wk.rearrange("p (dc c d) -> p dc c d", dc=2, c=EC)
    x_r = xT.rearrange("p (sc c s) -> p sc c s", sc=4, c=EC)
    wo_r = wo.rearrange("p (c e) -> p c e", c=2)
    nc.sync.dma_start(out=xT_t[:, 0, 0:4, :], in_=x_r[:, 0, 0:4, :])
    nc.sync.dma_start(out=xT_t[:, 0, 4:8, :], in_=x_r[:, 0, 4:8, :])
    for sc in range(1, 4):
        nc.sync.dma_start(out=xT_t[:, sc, :, :], in_=x_r[:, sc, :, :])
    nc.gpsimd.dma_start(out=bqk_t[:, :], in_=bqk)
    nc.gpsimd.dma_start(out=wk_t[:, 0, :, :], in_=wk_r[:, 0, :, :])
    nc.gpsimd.dma_start(out=wq_t[:, 0, :, :], in_=wq_r[:, 0, :, :])
    nc.gpsimd.dma_start(out=wv_t[:, :, :], in_=wv.rearrange(
        "p (c d) -> p c d", c=EC))
    for kc in range(KC):
        nc.vector.memset(v_t[:, kc, :, D:D + 1], 1.0)

    def qk_part(dc, proj, sc, half, state={}):
        # psum[d, s] += W[e, d].T @ X^T[e, s], two halves so bursts stay
        # small; bias folded into the DVE evacuation copy.
        w_t, dst = ((wq_t, qT_t), (wk_t, kT_t))[proj]
        if half == 0:
            state[(dc, proj, sc)] = ps_acc.tile(
                [128, 512], F32, tag="acc", name="ps_qk")
        ps = state[(dc, proj, sc)]
        ecs = range(EC // 2) if half == 0 else range(EC // 2, EC)
        for ec in ecs:
            nc.tensor.matmul(
                ps[:, :],
                lhsT=w_t[:, dc, ec, :],
                rhs=xT_t[:, sc, ec, :],
                start=(ec == 0), stop=(ec == EC - 1))
        if half == 1:
            nc.vector.tensor_scalar(
                out=dst[:, dc, sc * 512:(sc + 1) * 512], in0=ps[:, :],
                scalar1=bqk_t[:, 2 * proj + dc:2 * proj + dc + 1],
                scalar2=None, op0=ADD)
            del state[(dc, proj, sc)]

    def v_part(kc, half, state={}):
        # psum[s, d] += X^T[e, s].T @ Wv[e, d]
        if half == 0:
            state[kc] = ps_acc.tile([128, 512], F32, tag="acc", name="ps_v")
        ps = state[kc]
        sc, ko = kc // 4, (kc % 4) * 128
        ecs = range(EC // 2) if half == 0 else range(EC // 2, EC)
        for ec in ecs:
            nc.tensor.matmul(
                ps[:, 0:DC],
                lhsT=xT_t[:, sc, ec, ko:ko + 128],
                rhs=wv_t[:, ec, :],
                start=(ec == 0), stop=(ec == EC - 1))
        if half == 1:
            nc.vector.tensor_copy(
                out=v_t[:, kc, :, 0:D],
                in_=ps[:, 0:DC].rearrange("p (h d) -> p h d", h=GH))
            del state[kc]

    def attention_scores(qc, hc, kc):
        # Head pair (2*hc, 2*hc+1): head hp=0 on SBUF partitions 0-63, hp=1
        # on 64-127, so the two scores matmuls run as independent 64x128 PE
        # tiles and one ACTIVATE covers both heads' exp.
        sco = ps_sco.tile([128, 2, 512], F32, tag="sco", name="sco")
        for hp in range(2):
            po = hp * 64
            nc.tensor.matmul(
                sco[:, hp, :],
                lhsT=kT_t[po:po + 64, hc, kc * 128:(kc + 1) * 128],
                rhs=qT_t[po:po + 64, hc, qc * 512:(qc + 1) * 512],
                start=True, stop=True)
        pT = sb_p.tile([128, 2, 512], MM_DT)
        nc.scalar.activation(
            out=pT[:, :, :], in_=sco[:, :, :], func=EXP_FUNC,
            scale=float(SCALE))
        return pT

    def pv_alloc():
        return [ps_acc.tile([128, 512], F32, tag="acc", name=f"acc{j}")
                for j in range(2)]

    def pv_kc(accs, hc, pTs, kc):
        for hp in range(2):
            h = 2 * hc + hp
            nc.tensor.matmul(
                accs[hp][0:D + 1, :],
                lhsT=v_t[:, kc, h, :],
                rhs=pTs[kc][:, hp, :],
                start=(kc == 0), stop=(kc == KC - 1))

    def attention_norm(qc, hc, accs):
        for hp in range(2):
            po = hp * 64
            rs = sb_norm.tile([1, 512], F32, tag="rs")
            nc.vector.tensor_copy(out=rs[:, :], in_=accs[hp][D:D + 1, :])
            inv_r = sb_norm.tile([1, 512], F32, tag="inv")
            nc.vector.reciprocal_approx_fast(out=inv_r[:, :], in_=rs[:, :])
            brd = sb_norm.tile([64, 512], F32, tag="brd")
            nc.gpsimd.partition_broadcast(brd[:, :], inv_r[:, :])
            nc.vector.tensor_mul(
                o_t[po:po + 64, hc, qc * 512:(qc + 1) * 512],
                accs[hp][0:D, :],
                brd[:, :])

    def y_group(qc, ec, copy_eng=None):
        # psum[e, s] += Wo[c, e].T @ O^T[c, s] for chunk (ec, qc)
        yp = ps_acc.tile([128, 512], F32, tag="acc", name="yp")
        for cc in range(2):
            nc.tensor.matmul(
                yp[:, :],
                lhsT=wo_t[:, cc, ec * 128:(ec + 1) * 128],
                rhs=o_t[:, cc, qc * 512:(qc + 1) * 512],
                start=(cc == 0), stop=(cc == 1))
        ys = sb_y.tile([128, 512], F32)
        if copy_eng == "scalar":
            nc.scalar.copy(out=ys[:, :], in_=yp[:, :])
        else:
            nc.vector.tensor_copy(out=ys[:, :], in_=yp[:, :])
        nc.sync.dma_start(
            out=yT[ec * 128:(ec + 1) * 128, qc * 512:(qc + 1) * 512],
            in_=ys[:, :])

    # --- software-pipelined emission (= static engine program order) ----
    # The Tile scheduler orders instructions statically and every engine
    # executes its queue IN ORDER, so emission layout IS the schedule: a
    # not-ready instruction head-of-line-blocks its whole engine.  Per kc
    # slot the PE gets: the scores pair (feeds ACT), PV units per the
    # global PV plan, and fillers placed no earlier than their data.
    #
    # PV plan: block b's PV runs as "self" units (kc 0..5 in its own slots
    # 10..15, lag >= 2 behind exp) plus "leftover" units (kc 6..15
    # compressed into the next block's slots 0..7, norm right after), so
    # the final tail holds only 2 kc of PV + norm + y3.  Blocks 0-2 keep
    # the simple spread (block 0/1 are loaded with v/projection fillers).
    blocks = [(0, 0), (1, 0), (0, 1), (1, 1), (2, 0), (2, 1), (3, 0), (3, 1)]

    pts_of = [[] for _ in range(8)]
    accs_of = {}

    def pv_unit(b, kc):
        if kc == 0:
            accs_of[b] = pv_alloc()
        pv_kc(accs_of[b], blocks[b][1], pts_of[b], kc)
        if kc == KC - 1:
            attention_norm(blocks[b][0], blocks[b][1], accs_of.pop(b))

    # pv_sched[bi][slot] = list of (owner_block, kc) units.  Uniform: one
    # unit per slot — block b runs its own kc 0..5 in slots 10..15 (lag
    # >= 2 behind its exp stream) and kc 6..15 in the next block's slots
    # 0..9 (norm lands at slot 9); the last block self-runs kc 0..13 at
    # lag 2 so the tail holds only 2 PV units + norm + y3.
    pv_sched = [dict() for _ in range(8)]
    pv_sched[0] = {10 + k: [(0, k)] for k in range(6)}
    for bi in range(1, 7):
        sched = {s: [(bi - 1, 6 + s)] for s in range(10)}
        sched.update({10 + k: [(bi, k)] for k in range(6)})
        pv_sched[bi] = sched
    sched = {s: [(6, 6 + s)] for s in range(10)}
    for k in range(14):                      # self, lag 2
        sched.setdefault(2 + k, []).append((7, k))
    pv_sched[7] = sched

    def qk(dc, proj, sc, half):
        return lambda: qk_part(dc, proj, sc, half)

    def vp(kc, half):
        return lambda: v_part(kc, half)

    def yg(qc, ec):
        return lambda: y_group(qc, ec)

    # fillers per block: (min_slot, fn); deadlines as emission-order
    # constraints (data must be emitted before its consumer's slot):
    #   K(dc0,s) before block0 kc=4s; Q(dc0,s1) < blk1; K/Q(dc1,s0) < blk2;
    #   K(dc1,s2/s3) before blk2 kc8/kc12; Q(dc1,s1) < blk3; Q(dc0,s2) <
    #   blk4; Q(dc1,s2) < blk5; Q(dc0,s3) < blk6; Q(dc1,s3) < blk7;
    #   v(kc) before its first PV unit; y(qc) after norm(qc,1); min_slots
    #   on early-block items also track DMA arrival order.
    fillers = {
        0: [(2, qk(0, 1, 1, 0)), (3, qk(0, 1, 1, 1)), (4, qk(0, 0, 1, 0)),
            (5, qk(0, 1, 2, 0)), (6, qk(0, 1, 2, 1)), (7, qk(0, 0, 1, 1)),
            (9, qk(0, 1, 3, 0)), (10, qk(0, 1, 3, 1))] +
           [(2 + k, vp(k, 0)) for k in range(10)] +
           [(3 + k, vp(k, 1)) for k in range(10)],
        1: [(0, vp(10, 0)), (1, vp(10, 1)), (1, vp(11, 0)), (2, vp(11, 1)),
            (3, vp(12, 0)), (4, vp(12, 1)), (4, vp(13, 0)), (5, vp(13, 1)),
            (6, vp(14, 0)), (7, vp(14, 1)), (7, vp(15, 0)), (8, vp(15, 1)),
            (9, qk(1, 1, 0, 0)), (10, qk(1, 1, 0, 1)),
            (11, qk(1, 0, 0, 0)), (12, qk(1, 0, 0, 1))],
        2: [(0, qk(1, 1, 1, 0)), (1, qk(1, 1, 1, 1)),
            (2, qk(1, 1, 2, 0)), (3, qk(1, 1, 2, 1)),
            (6, qk(1, 1, 3, 0)), (7, qk(1, 1, 3, 1)),
            (10, qk(1, 0, 1, 0)), (11, qk(1, 0, 1, 1))],
        3: [(0, qk(0, 0, 2, 0)), (1, qk(0, 0, 2, 1))],
        4: [(0, qk(1, 0, 2, 0)), (1, qk(1, 0, 2, 1))] +
           [(0, yg(0, e)) for e in range(EC)],
        5: [(0, qk(0, 0, 3, 0)), (1, qk(0, 0, 3, 1))] +
           [(0, yg(1, e)) for e in range(EC)],
        6: [(0, qk(1, 0, 3, 0)), (1, qk(1, 0, 3, 1))] +
           [(11, yg(2, e)) for e in range(EC)],
        7: [],
    }

    def layout(items):
        # first-fit at the intended slot, spill forward once 2 items deep
        slots = [[] for _ in range(16)]
        for ms, fn in items:
            s = ms
            while s < 15 and len(slots[s]) >= 2:
                s += 1
            slots[s].append(fn)
        return slots

    # block 0's own first tiles; h0 halves first so they start as soon as
    # the low e-chunks of x(sc0) land, h1 halves follow the high chunks
    qk_part(0, 0, 0, 0)
    qk_part(0, 1, 0, 0)
    qk_part(0, 0, 0, 1)
    qk_part(0, 1, 0, 1)
    # gate C (first qT tile copied, ~20us): releases dc1 weights + wo
    nc.vector.tensor_mul(gate_t[0:1, 48:56], qT_t[0:1, 0, 0:8],
                         wq_t[0:1, 1, 0, 0:8])
    nc.vector.tensor_mul(gate_t[0:1, 56:64], qT_t[0:1, 0, 0:8],
                         wk_t[0:1, 1, 0, 0:8])
    nc.vector.tensor_mul(gate_t[0:1, 64:72], qT_t[0:1, 0, 0:8],
                         wo_t[0:1, 0, 0:8])
    nc.vector.tensor_mul(gate_t[0:1, 72:80], qT_t[0:1, 0, 0:8],
                         wo_t[0:1, 1, 0:8])
    nc.gpsimd.dma_start(out=wq_t[:, 1, :, :], in_=wq_r[:, 1, :, :])
    nc.gpsimd.dma_start(out=wk_t[:, 1, :, :], in_=wk_r[:, 1, :, :])
    nc.scalar.dma_start(out=wo_t[:, 0, :], in_=wo_r[:, 0, :])
    nc.scalar.dma_start(out=wo_t[:, 1, :], in_=wo_r[:, 1, :])

    for bi, (qc, hc) in enumerate(blocks):
        fl_slots = layout(fillers.get(bi, []))
        for kc in range(0, KC, 2):
            # scores pairs back-to-back: the second pair's kT LDWEIGHTS
            # overlaps the first pair's matmuls (disjoint PE row groups)
            pts_of[bi].append(attention_scores(qc, hc, kc))
            pts_of[bi].append(attention_scores(qc, hc, kc + 1))
            for s in (kc, kc + 1):
                for fn in fl_slots[s]:
                    fn()
                for b, k in pv_sched[bi].get(s, []):
                    pv_unit(b, k)
    # tail: last 2 PV units + norm + the last y chunks
    pv_unit(7, 14)
    pv_unit(7, 15)
    for ec in range(EC):
        y_group(3, ec, copy_eng="scalar" if ec % 2 else None)

    if dbg is not None:
        for name, t in (("qT", qT_t), ("kT", kT_t), ("o", o_t)):
            f = sb_big.tile([128, 2, S], F32, name=f"dump_{name}")
            nc.vector.tensor_copy(out=f[:, :, :], in_=t[:, :, :])
            nc.sync.dma_start(out=dbg[name], in_=f.rearrange("p a b -> p (a b)"))
        fv = sb_big.tile([128, KC, GH, D + 1], F32, name="dump_v")
        nc.vector.tensor_copy(out=fv[:, :, :, :], in_=v_t[:, :, :, :])
        nc.sync.dma_start(out=dbg["v"], in_=fv.rearrange("p a b c -> p (a b c)"))


_cached_nc = None


def _build():
    nc = bacc.Bacc(trn_type="TRN2", target_bir_lowering=False)
    xT = nc.dram_tensor("xT", [128, EC * S], MM_DT, kind="ExternalInput").ap()
    wq = nc.dram_tensor("wq", [128, EC * DC], MM_DT, kind="ExternalInput").ap()
    wk = nc.dram_tensor("wk", [128, EC * DC], MM_DT, kind="ExternalInput").ap()
    wv = nc.dram_tensor("wv", [128, EC * DC], MM_DT, kind="ExternalInput").ap()
    wo = nc.dram_tensor("wo", [128, 2 * E], MM_DT, kind="ExternalInput").ap()
    bqk = nc.dram_tensor("bqk", [128, 4], F32, kind="ExternalInput").ap()
    yT = nc.dram_tensor("yT", [E, S], F32, kind="ExternalOutput").ap()
    dbg = None
    if DEBUG_DUMPS:
        dbg = {
            "qT": nc.dram_tensor("dbg_qT", [128, 2 * S], F32, kind="ExternalOutput").ap(),
            "kT": nc.dram_tensor("dbg_kT", [128, 2 * S], F32, kind="ExternalOutput").ap(),
            "o": nc.dram_tensor("dbg_o", [128, 2 * S], F32, kind="ExternalOutput").ap(),
            "v": nc.dram_tensor("dbg_v", [128, KC * GH * (D + 1)], F32, kind="ExternalOutput").ap(),
        }
    with tile.TileContext(nc) as tc:
        with ExitStack() as ctx:
            _emit(nc, tc, ctx, xT, wq, wk, wv, wo, bqk, yT, dbg)
    nc.compile()
    return nc


def get_nc():
    global _cached_nc
    if _cached_nc is None:
        _cached_nc = _build()
    return _cached_nc


def make_in_maps(inputs, wq, bq, wk, bk, wv, wo):
    in_maps = []
    for c in range(NCORES):
        b, g = divmod(c, GH)
        sl = slice(g * DC, (g + 1) * DC)
        def perm(a):
            # [C*128, N] -> [128, C*N] with SBUF chunk-major free dim
            cN = a.shape[0] // 128
            return np.ascontiguousarray(
                a.reshape(cN, 128, a.shape[1]).transpose(1, 0, 2).reshape(
                    128, cN * a.shape[1]))

        def perm_x(a):
            # [E, S] -> [128, SC, EC, 512] flattened (sc-major, 8KB runs)
            return np.ascontiguousarray(
                a.reshape(EC, 128, 4, 512).transpose(1, 2, 0, 3).reshape(
                    128, -1))

        def perm_w(a):
            # [E, DC] -> [128, 2, EC, 128] flattened (dc-major, 2KB runs)
            return np.ascontiguousarray(
                a.reshape(EC, 128, 2, 128).transpose(1, 2, 0, 3).reshape(
                    128, -1))

        bq_g, bk_g = bq[sl], bk[sl]
        bqk = np.stack([bq_g[0:128], bq_g[128:256],
                        bk_g[0:128], bk_g[128:256]], axis=1)
        in_maps.append({
            "xT": round_f32r(perm_x(np.ascontiguousarray(inputs[b].T))),
            "wq": round_f32r(perm_w(wq[:, sl])),
            "wk": round_f32r(perm_w(wk[:, sl])),
            "wv": round_f32r(perm(wv[:, sl])),
            "wo": round_f32r(perm(wo[sl, :])),
            "bqk": np.ascontiguousarray(bqk, np.float32),
        })
    return in_maps


def combine(results, wv_full, bv, wo_full, bo):
    y = np.zeros((B, S, E), np.float32)
    for c in range(NCORES):
        y[c // GH] += results[c]["yT"].T
    y += bv @ wo_full + bo
    return y


def kernel(inputs, wq, bq, wk, bk, wv, bv, wo, bo, _run_kwargs=None):
    inputs = np.asarray(inputs, np.float32)
    wq, bq = np.asarray(wq, np.float32), np.asarray(bq, np.float32)
    wk, bk = np.asarray(wk, np.float32), np.asarray(bk, np.float32)
    wv, bv = np.asarray(wv, np.float32), np.asarray(bv, np.float32)
    wo, bo = np.asarray(wo, np.float32), np.asarray(bo, np.float32)

    nc = get_nc()
    in_maps = make_in_maps(inputs, wq, bq, wk, bk, wv, wo)
    res = run_bass_kernel_spmd(nc, in_maps, list(range(NCORES)),
                               **(_run_kwargs or {}))
    y = combine(res.results, wv, bv, wo, bo)
    if _run_kwargs:
        kernel.last_result = res
    return y
